# revision 4
# baseline (speedup 1.0000x reference)
"""RNN-T JointNetwork Trainium2 kernel (8-core data-parallel over batch B).

Reference computation (per batch b):
    e[t, j] = sum_d h_enc[b,t,0,d] * W_enc[j,d] + b_enc[j]        (256 x 512)
    d[u, j] = sum_d h_dec[b,0,u,d] * W_dec[j,d]                   (64 x 512)
    z[t,u,j] = tanh(e[t,j] + d[u,j])                              (256 x 64 x 512)
    out[t,u,v] = sum_j z[t,u,j] * W_out[v,j] + b_out[v]           (256 x 64 x 1024)

Sharding: one batch element per NeuronCore (B == 8 == n_cores).

On-chip layout (everything j/d-partitioned so matmuls contract partitions):
    e_T  (j, t)  in SBUF  -- produced by PE from host-transposed henc/wenc
    d_T  (j, u)  in SBUF
    z_u  (j, t) bf16 for one u  -- ACT: tanh(e_T + d_T[:,u]) (bias = per-partition)
    main matmul: psum[t', v] += z_u[:, jc, t']^T @ W_outT[jc][:, v]  (bf16)
    DVE adds b_out (broadcast tile) while evacuating PSUM -> SBUF -> DMA out.
"""

import numpy as np
import ml_dtypes

B, T, U = 8, 256, 64
D = 512
J = 512
V = 1024
JC = J // 128  # 4 chunks of the contraction/output-j dim
DC = D // 128  # 4 chunks of the input-d dim
TH = T // 128  # 2 t-halves per u group

_compiled = None  # (nc, input_names) cache


def _patch_walrus_ldw_opt():
    """The stock compile line passes --enable-ldw-opt=false; with one
    LDWEIGHTS emitted per MATMUL (repeated lhsT never deduped) that leaves
    a few us of weight-load exposure on the PE stream. Flip it on."""
    import concourse.bass_utils as bu

    if getattr(bu.run_command, "_ldw_patched", False):
        return
    orig = bu.run_command

    def patched(argv, **kwargs):
        if isinstance(argv, list):
            argv = [
                "--enable-ldw-opt=true" if a == "--enable-ldw-opt=false" else a
                for a in argv
            ]
        return orig(argv, **kwargs)

    patched._ldw_patched = True
    bu.run_command = patched


def _build_kernel():
    import concourse.bass as bass
    import concourse.tile as tile
    import concourse.mybir as mybir

    # NOTE: tried --enable-ldw-opt=true to dedupe repeated-lhsT LDWEIGHTS;
    # walrus codegen rejects it (visitInstLdweights, fp32 weight loads).
    fp32 = mybir.dt.float32
    bf16 = mybir.dt.bfloat16

    nc = bass.Bass("TRN2", target_bir_lowering=False, debug=False)

    henc = nc.dram_tensor("henc", [128, DC, T], fp32, kind="ExternalInput")
    hdec = nc.dram_tensor("hdec", [128, DC, U], fp32, kind="ExternalInput")
    wenc = nc.dram_tensor("wenc", [128, DC, JC, 128], fp32, kind="ExternalInput")
    wdec = nc.dram_tensor("wdec", [128, DC, JC, 128], fp32, kind="ExternalInput")
    wout = nc.dram_tensor("wout", [128, JC, V], bf16, kind="ExternalInput")
    benc = nc.dram_tensor("benc", [128, JC], fp32, kind="ExternalInput")
    bout = nc.dram_tensor("bout", [128, V], fp32, kind="ExternalInput")
    out = nc.dram_tensor("out", [T, U, V], fp32, kind="ExternalOutput")

    with tile.TileContext(nc) as tc:
        with (
            tc.tile_pool(name="const", bufs=1) as cpool,
            tc.tile_pool(name="work", bufs=1) as wpool,
            tc.tile_pool(name="z", bufs=3) as zpool,
            tc.tile_pool(name="o", bufs=6) as opool,
            tc.tile_pool(name="pse", bufs=2, space="PSUM") as psepool,
            tc.tile_pool(name="ps", bufs=3, space="PSUM") as pspool,
        ):
            # e-path inputs on SWDGE (gpsimd), split per-dc so the first
            # encoder matmul fires after ~400KB instead of 4.2MB.
            benc_sb = cpool.tile([128, JC], fp32, tag="benc")
            nc.gpsimd.dma_start(benc_sb[:], benc[:])
            henc_sb = cpool.tile([128, DC, T], fp32, tag="henc")
            wenc_sb = cpool.tile([128, DC, JC, 128], fp32, tag="wenc")
            for dc in range(DC):
                nc.gpsimd.dma_start(wenc_sb[:, dc, :, :], wenc[:, dc, :, :])
                nc.gpsimd.dma_start(henc_sb[:, dc, :], henc[:, dc, :])
            # d-path inputs on the ACT HWDGE ring (parallel with SWDGE)
            wdec_sb = cpool.tile([128, DC, JC, 128], fp32, tag="wdec")
            nc.scalar.dma_start(wdec_sb[:], wdec[:])
            hdec_sb = cpool.tile([128, DC, U], fp32, tag="hdec")
            nc.scalar.dma_start(hdec_sb[:], hdec[:])
            # main-loop weights on the SP HWDGE ring (parallel with both)
            wout_sb = cpool.tile([128, JC, V], bf16, tag="wout")
            nc.sync.dma_start(wout_sb[:], wout[:])
            bout_sb = cpool.tile([128, V], fp32, tag="bout")
            nc.sync.dma_start(bout_sb[:], bout[:])

            # --- prologue: e_T (j, t) and d_T (j, u) ---
            e_sb = wpool.tile([128, JC, T], fp32, tag="e")
            d_sb = wpool.tile([128, JC, U], fp32, tag="d")
            for jc in range(JC):
                ps_e = psepool.tile([128, T], fp32, tag="pse")
                for dc in range(DC):
                    nc.tensor.matmul(
                        ps_e[:],
                        wenc_sb[:, dc, jc, :],
                        henc_sb[:, dc, :],
                        start=(dc == 0),
                        stop=(dc == DC - 1),
                    )
                # evacuate with the encoder bias folded in (per-partition bias)
                nc.scalar.activation(
                    e_sb[:, jc, :], ps_e[:],
                    mybir.ActivationFunctionType.Identity,
                    bias=benc_sb[:, jc : jc + 1],
                )
            for jc in range(JC):
                ps_d = psepool.tile([128, U], fp32, tag="pse")
                for dc in range(DC):
                    nc.tensor.matmul(
                        ps_d[:],
                        wdec_sb[:, dc, jc, :],
                        hdec_sb[:, dc, :],
                        start=(dc == 0),
                        stop=(dc == DC - 1),
                    )
                nc.vector.tensor_copy(d_sb[:, jc, :], ps_d[:])

            # --- main loop: one u at a time ---
            for u in range(U):
                z_u = zpool.tile([128, JC, T], bf16, tag="z")
                for jc in range(JC):
                    # z = tanh(e + d_u); d_u enters as the per-partition bias
                    nc.scalar.activation(
                        z_u[:, jc, :], e_sb[:, jc, :],
                        mybir.ActivationFunctionType.Tanh,
                        bias=d_sb[:, jc, u : u + 1],
                    )
                for th in range(TH):
                    ps = pspool.tile([128, V], fp32, tag="ps")
                    for jc in range(JC):
                        lhsT = z_u[:, jc, th * 128 : (th + 1) * 128]
                        nc.tensor.matmul(
                            ps[:, 0:512], lhsT, wout_sb[:, jc, 0:512],
                            start=(jc == 0), stop=(jc == JC - 1),
                        )
                        nc.tensor.matmul(
                            ps[:, 512:1024], lhsT, wout_sb[:, jc, 512:1024],
                            start=(jc == 0), stop=(jc == JC - 1),
                        )
                    ot = opool.tile([128, V], fp32, tag="ot")
                    nc.vector.tensor_add(ot[:], ps[:], bout_sb[:])
                    nc.sync.dma_start(out[th * 128 : (th + 1) * 128, u, :], ot[:])

    _split_multiwait(nc)
    return nc


def _split_multiwait(nc, max_waits=1):
    """The pinned walrus rejects instructions carrying >~2 sem waits
    ("Too many sync wait commands"). Split extras into single-wait NoOps
    executed immediately before the offending instruction on the same engine."""
    import concourse.mybir as mybir

    for fn in nc.m.functions:
        for blk in fn.blocks:
            newlist = []
            for inst in blk.instructions:
                si = getattr(inst, "sync_info", None)
                if si is not None and si.on_wait and len(si.on_wait) > max_waits:
                    waits = list(si.on_wait)
                    for j, w in enumerate(waits[max_waits:]):
                        nop = mybir.InstNoOp(
                            name=f"{inst.name}-ws{j}", ins=[], outs=[],
                            sync_info=mybir.SyncInfo(on_wait=[w], on_update=[]),
                        )
                        nop.engine = inst.engine
                        newlist.append(nop)
                    si.on_wait = waits[:max_waits]
                newlist.append(inst)
            blk.instructions = newlist


def _prep_inputs(h_enc, h_dec, W_enc, b_enc, W_dec, W_out, b_out):
    """Host-side relayout into the SBUF layouts the kernel expects."""
    f32 = np.float32
    h_enc = np.asarray(h_enc, f32)
    h_dec = np.asarray(h_dec, f32)
    W_enc = np.asarray(W_enc, f32)
    W_dec = np.asarray(W_dec, f32)
    W_out = np.asarray(W_out, f32)
    b_enc = np.asarray(b_enc, f32)
    b_out = np.asarray(b_out, f32)

    wenc = np.ascontiguousarray(
        W_enc.reshape(JC, 128, DC, 128).transpose(3, 2, 0, 1))  # [p, dc, jc, m]
    wdec = np.ascontiguousarray(
        W_dec.reshape(JC, 128, DC, 128).transpose(3, 2, 0, 1))
    wout = np.ascontiguousarray(
        W_out.reshape(V, JC, 128).transpose(2, 1, 0)).astype(ml_dtypes.bfloat16)
    benc = np.ascontiguousarray(b_enc.reshape(JC, 128).T)
    bout = np.ascontiguousarray(np.broadcast_to(b_out[None, :], (128, V)))

    in_maps = []
    for b in range(B):
        A = h_enc[b, :, 0, :]  # (T, D)
        henc = np.ascontiguousarray(A.reshape(T, DC, 128).transpose(2, 1, 0))
        Bm = h_dec[b, 0, :, :]  # (U, D)
        hdec = np.ascontiguousarray(Bm.reshape(U, DC, 128).transpose(2, 1, 0))
        in_maps.append({
            "henc": henc, "hdec": hdec,
            "wenc": wenc, "wdec": wdec, "wout": wout,
            "benc": benc, "bout": bout,
        })
    return in_maps


def run(inputs, trace=False, trace_cores=None):
    """Build+compile (cached), run on 8 cores, return (output, BassKernelResults)."""
    from concourse.bass_utils import run_bass_kernel_spmd

    global _compiled
    if _compiled is None:
        _compiled = _build_kernel()
    nc = _compiled

    in_maps = _prep_inputs(**inputs)
    res = run_bass_kernel_spmd(
        nc, in_maps, core_ids=list(range(B)), trace=trace,
        trace_cores=trace_cores,
    )
    outp = np.stack([np.asarray(res.results[b]["out"]) for b in range(B)], axis=0)
    return outp.astype(np.float32, copy=False), res


def kernel(**inputs):
    outp, _ = run(inputs, trace=False)
    return outp


# revision 5
# speedup vs baseline: 1.0121x; 1.0121x over previous
"""RNN-T JointNetwork Trainium2 kernel (8-core data-parallel over batch B).

Reference computation (per batch b):
    e[t, j] = sum_d h_enc[b,t,0,d] * W_enc[j,d] + b_enc[j]        (256 x 512)
    d[u, j] = sum_d h_dec[b,0,u,d] * W_dec[j,d]                   (64 x 512)
    z[t,u,j] = tanh(e[t,j] + d[u,j])                              (256 x 64 x 512)
    out[t,u,v] = sum_j z[t,u,j] * W_out[v,j] + b_out[v]           (256 x 64 x 1024)

Sharding: one batch element per NeuronCore (B == 8 == n_cores).

On-chip layout (everything j/d-partitioned so matmuls contract partitions):
    e_T  (j, t)  in SBUF  -- produced by PE from host-transposed henc/wenc
    d_T  (j, u)  in SBUF
    z_u  (j, t) bf16 for one u  -- ACT: tanh(e_T + d_T[:,u]) (bias = per-partition)
    main matmul: psum[t', v] += z_u[:, jc, t']^T @ W_outT[jc][:, v]  (bf16)
    DVE adds b_out (broadcast tile) while evacuating PSUM -> SBUF -> DMA out.
"""

import numpy as np
import ml_dtypes

B, T, U = 8, 256, 64
D = 512
J = 512
V = 1024
JC = J // 128  # 4 chunks of the contraction/output-j dim
DC = D // 128  # 4 chunks of the input-d dim
TH = T // 128  # 2 t-halves per u group

_compiled = None  # (nc, input_names) cache


def _patch_walrus_ldw_opt():
    """The stock compile line passes --enable-ldw-opt=false; with one
    LDWEIGHTS emitted per MATMUL (repeated lhsT never deduped) that leaves
    a few us of weight-load exposure on the PE stream. Flip it on."""
    import concourse.bass_utils as bu

    if getattr(bu.run_command, "_ldw_patched", False):
        return
    orig = bu.run_command

    def patched(argv, **kwargs):
        if isinstance(argv, list):
            argv = [
                "--enable-ldw-opt=true" if a == "--enable-ldw-opt=false" else a
                for a in argv
            ]
        return orig(argv, **kwargs)

    patched._ldw_patched = True
    bu.run_command = patched


def _build_kernel():
    import concourse.bass as bass
    import concourse.tile as tile
    import concourse.mybir as mybir

    # NOTE: tried --enable-ldw-opt=true to dedupe repeated-lhsT LDWEIGHTS;
    # walrus codegen rejects it (visitInstLdweights, fp32 weight loads).
    fp32 = mybir.dt.float32
    bf16 = mybir.dt.bfloat16

    nc = bass.Bass("TRN2", target_bir_lowering=False, debug=False)

    henc = nc.dram_tensor("henc", [128, DC, T], fp32, kind="ExternalInput")
    hdec = nc.dram_tensor("hdec", [128, DC, U], fp32, kind="ExternalInput")
    wenc = nc.dram_tensor("wenc", [128, DC, JC, 128], fp32, kind="ExternalInput")
    wdec = nc.dram_tensor("wdec", [128, DC, JC, 128], fp32, kind="ExternalInput")
    wout = nc.dram_tensor("wout", [128, JC, V], bf16, kind="ExternalInput")
    benc = nc.dram_tensor("benc", [128, JC], fp32, kind="ExternalInput")
    bout = nc.dram_tensor("bout", [128, V], fp32, kind="ExternalInput")
    out = nc.dram_tensor("out", [T, U, V], fp32, kind="ExternalOutput")

    with tile.TileContext(nc) as tc:
        with (
            tc.tile_pool(name="const", bufs=1) as cpool,
            tc.tile_pool(name="work", bufs=1) as wpool,
            tc.tile_pool(name="z", bufs=3) as zpool,
            tc.tile_pool(name="o", bufs=6) as opool,
            tc.tile_pool(name="pse", bufs=2, space="PSUM") as psepool,
            tc.tile_pool(name="ps", bufs=3, space="PSUM") as pspool,
        ):
            # All loads on the two HWDGE rings (RTL descriptor gen; SWDGE's
            # Q7 descriptor generation costs ~2.5us *per dma_start* and was
            # delaying the first matmul to ~18us). e-path pieces first so
            # PE starts matmuls (and warms the HAM clock gate) at ~3us
            # while the remaining loads stream in parallel.
            benc_sb = cpool.tile([128, JC], fp32, tag="benc")
            nc.scalar.dma_start(benc_sb[:], benc[:])
            henc_sb = cpool.tile([128, DC, T], fp32, tag="henc")
            wenc_sb = cpool.tile([128, DC, JC, 128], fp32, tag="wenc")
            for dc in range(DC):
                nc.scalar.dma_start(wenc_sb[:, dc, :, :], wenc[:, dc, :, :])
                nc.scalar.dma_start(henc_sb[:, dc, :], henc[:, dc, :])
            # d-path inputs follow on the same ring
            wdec_sb = cpool.tile([128, DC, JC, 128], fp32, tag="wdec")
            nc.scalar.dma_start(wdec_sb[:], wdec[:])
            hdec_sb = cpool.tile([128, DC, U], fp32, tag="hdec")
            nc.scalar.dma_start(hdec_sb[:], hdec[:])
            # main-loop weights on the SP HWDGE ring (parallel with ACT ring)
            wout_sb = cpool.tile([128, JC, V], bf16, tag="wout")
            nc.sync.dma_start(wout_sb[:], wout[:])
            bout_sb = cpool.tile([128, V], fp32, tag="bout")
            nc.sync.dma_start(bout_sb[:], bout[:])

            # --- prologue: e_T (j, t) and d_T (j, u) ---
            e_sb = wpool.tile([128, JC, T], fp32, tag="e")
            d_sb = wpool.tile([128, JC, U], fp32, tag="d")
            for jc in range(JC):
                ps_e = psepool.tile([128, T], fp32, tag="pse")
                for dc in range(DC):
                    nc.tensor.matmul(
                        ps_e[:],
                        wenc_sb[:, dc, jc, :],
                        henc_sb[:, dc, :],
                        start=(dc == 0),
                        stop=(dc == DC - 1),
                    )
                # evacuate with the encoder bias folded in (per-partition bias)
                nc.scalar.activation(
                    e_sb[:, jc, :], ps_e[:],
                    mybir.ActivationFunctionType.Identity,
                    bias=benc_sb[:, jc : jc + 1],
                )
            for jc in range(JC):
                ps_d = psepool.tile([128, U], fp32, tag="pse")
                for dc in range(DC):
                    nc.tensor.matmul(
                        ps_d[:],
                        wdec_sb[:, dc, jc, :],
                        hdec_sb[:, dc, :],
                        start=(dc == 0),
                        stop=(dc == DC - 1),
                    )
                nc.vector.tensor_copy(d_sb[:, jc, :], ps_d[:])

            # --- main loop: one u at a time ---
            for u in range(U):
                z_u = zpool.tile([128, JC, T], bf16, tag="z")
                for jc in range(JC):
                    # z = tanh(e + d_u); d_u enters as the per-partition bias
                    nc.scalar.activation(
                        z_u[:, jc, :], e_sb[:, jc, :],
                        mybir.ActivationFunctionType.Tanh,
                        bias=d_sb[:, jc, u : u + 1],
                    )
                for th in range(TH):
                    ps = pspool.tile([128, V], fp32, tag="ps")
                    for jc in range(JC):
                        lhsT = z_u[:, jc, th * 128 : (th + 1) * 128]
                        nc.tensor.matmul(
                            ps[:, 0:512], lhsT, wout_sb[:, jc, 0:512],
                            start=(jc == 0), stop=(jc == JC - 1),
                        )
                        nc.tensor.matmul(
                            ps[:, 512:1024], lhsT, wout_sb[:, jc, 512:1024],
                            start=(jc == 0), stop=(jc == JC - 1),
                        )
                    ot = opool.tile([128, V], fp32, tag="ot")
                    nc.vector.tensor_add(ot[:], ps[:], bout_sb[:])
                    nc.sync.dma_start(out[th * 128 : (th + 1) * 128, u, :], ot[:])

    _split_multiwait(nc)
    return nc


def _split_multiwait(nc, max_waits=1):
    """The pinned walrus rejects instructions carrying >~2 sem waits
    ("Too many sync wait commands"). Split extras into single-wait NoOps
    executed immediately before the offending instruction on the same engine."""
    import concourse.mybir as mybir

    for fn in nc.m.functions:
        for blk in fn.blocks:
            newlist = []
            for inst in blk.instructions:
                si = getattr(inst, "sync_info", None)
                if si is not None and si.on_wait and len(si.on_wait) > max_waits:
                    waits = list(si.on_wait)
                    for j, w in enumerate(waits[max_waits:]):
                        nop = mybir.InstNoOp(
                            name=f"{inst.name}-ws{j}", ins=[], outs=[],
                            sync_info=mybir.SyncInfo(on_wait=[w], on_update=[]),
                        )
                        nop.engine = inst.engine
                        newlist.append(nop)
                    si.on_wait = waits[:max_waits]
                newlist.append(inst)
            blk.instructions = newlist


def _prep_inputs(h_enc, h_dec, W_enc, b_enc, W_dec, W_out, b_out):
    """Host-side relayout into the SBUF layouts the kernel expects."""
    f32 = np.float32
    h_enc = np.asarray(h_enc, f32)
    h_dec = np.asarray(h_dec, f32)
    W_enc = np.asarray(W_enc, f32)
    W_dec = np.asarray(W_dec, f32)
    W_out = np.asarray(W_out, f32)
    b_enc = np.asarray(b_enc, f32)
    b_out = np.asarray(b_out, f32)

    wenc = np.ascontiguousarray(
        W_enc.reshape(JC, 128, DC, 128).transpose(3, 2, 0, 1))  # [p, dc, jc, m]
    wdec = np.ascontiguousarray(
        W_dec.reshape(JC, 128, DC, 128).transpose(3, 2, 0, 1))
    wout = np.ascontiguousarray(
        W_out.reshape(V, JC, 128).transpose(2, 1, 0)).astype(ml_dtypes.bfloat16)
    benc = np.ascontiguousarray(b_enc.reshape(JC, 128).T)
    bout = np.ascontiguousarray(np.broadcast_to(b_out[None, :], (128, V)))

    in_maps = []
    for b in range(B):
        A = h_enc[b, :, 0, :]  # (T, D)
        henc = np.ascontiguousarray(A.reshape(T, DC, 128).transpose(2, 1, 0))
        Bm = h_dec[b, 0, :, :]  # (U, D)
        hdec = np.ascontiguousarray(Bm.reshape(U, DC, 128).transpose(2, 1, 0))
        in_maps.append({
            "henc": henc, "hdec": hdec,
            "wenc": wenc, "wdec": wdec, "wout": wout,
            "benc": benc, "bout": bout,
        })
    return in_maps


def run(inputs, trace=False, trace_cores=None):
    """Build+compile (cached), run on 8 cores, return (output, BassKernelResults)."""
    from concourse.bass_utils import run_bass_kernel_spmd

    global _compiled
    if _compiled is None:
        _compiled = _build_kernel()
    nc = _compiled

    in_maps = _prep_inputs(**inputs)
    res = run_bass_kernel_spmd(
        nc, in_maps, core_ids=list(range(B)), trace=trace,
        trace_cores=trace_cores,
    )
    outp = np.stack([np.asarray(res.results[b]["out"]) for b in range(B)], axis=0)
    return outp.astype(np.float32, copy=False), res


def kernel(**inputs):
    outp, _ = run(inputs, trace=False)
    return outp


# revision 6
# speedup vs baseline: 1.0130x; 1.0010x over previous
"""RNN-T JointNetwork Trainium2 kernel (8-core data-parallel over batch B).

Reference computation (per batch b):
    e[t, j] = sum_d h_enc[b,t,0,d] * W_enc[j,d] + b_enc[j]        (256 x 512)
    d[u, j] = sum_d h_dec[b,0,u,d] * W_dec[j,d]                   (64 x 512)
    z[t,u,j] = tanh(e[t,j] + d[u,j])                              (256 x 64 x 512)
    out[t,u,v] = sum_j z[t,u,j] * W_out[v,j] + b_out[v]           (256 x 64 x 1024)

Sharding: one batch element per NeuronCore (B == 8 == n_cores).

On-chip layout (everything j/d-partitioned so matmuls contract partitions):
    e_T  (j, t)  in SBUF  -- produced by PE from host-transposed henc/wenc
    d_T  (j, u)  in SBUF
    z_u  (j, t) bf16 for one u  -- ACT: tanh(e_T + d_T[:,u]) (bias = per-partition)
    main matmul: psum[t', v] += z_u[:, jc, t']^T @ W_outT[jc][:, v]  (bf16)
    DVE adds b_out (broadcast tile) while evacuating PSUM -> SBUF -> DMA out.
"""

import numpy as np
import ml_dtypes

B, T, U = 8, 256, 64
D = 512
J = 512
V = 1024
JC = J // 128  # 4 chunks of the contraction/output-j dim
DC = D // 128  # 4 chunks of the input-d dim
TH = T // 128  # 2 t-halves per u group

_compiled = None  # (nc, input_names) cache


def _patch_walrus_ldw_opt():
    """The stock compile line passes --enable-ldw-opt=false; with one
    LDWEIGHTS emitted per MATMUL (repeated lhsT never deduped) that leaves
    a few us of weight-load exposure on the PE stream. Flip it on."""
    import concourse.bass_utils as bu

    if getattr(bu.run_command, "_ldw_patched", False):
        return
    orig = bu.run_command

    def patched(argv, **kwargs):
        if isinstance(argv, list):
            argv = [
                "--enable-ldw-opt=true" if a == "--enable-ldw-opt=false" else a
                for a in argv
            ]
        return orig(argv, **kwargs)

    patched._ldw_patched = True
    bu.run_command = patched


def _build_kernel():
    import concourse.bass as bass
    import concourse.tile as tile
    import concourse.mybir as mybir

    # NOTE: tried --enable-ldw-opt=true to dedupe repeated-lhsT LDWEIGHTS;
    # walrus codegen rejects it (visitInstLdweights, fp32 weight loads).
    fp32 = mybir.dt.float32
    bf16 = mybir.dt.bfloat16

    nc = bass.Bass("TRN2", target_bir_lowering=False, debug=False)

    henc = nc.dram_tensor("henc", [128, DC, T], fp32, kind="ExternalInput")
    hdec = nc.dram_tensor("hdec", [128, DC, U], fp32, kind="ExternalInput")
    wenc = nc.dram_tensor("wenc", [128, DC, JC, 128], fp32, kind="ExternalInput")
    wdec = nc.dram_tensor("wdec", [128, DC, JC, 128], fp32, kind="ExternalInput")
    wout = nc.dram_tensor("wout", [128, JC, V], bf16, kind="ExternalInput")
    benc = nc.dram_tensor("benc", [128, JC], fp32, kind="ExternalInput")
    bout = nc.dram_tensor("bout", [128, V], fp32, kind="ExternalInput")
    out = nc.dram_tensor("out", [T, U, V], fp32, kind="ExternalOutput")

    with tile.TileContext(nc) as tc:
        with (
            tc.tile_pool(name="const", bufs=1) as cpool,
            tc.tile_pool(name="work", bufs=1) as wpool,
            tc.tile_pool(name="z", bufs=3) as zpool,
            tc.tile_pool(name="o", bufs=6) as opool,
            tc.tile_pool(name="pse", bufs=2, space="PSUM") as psepool,
            tc.tile_pool(name="ps", bufs=3, space="PSUM") as pspool,
        ):
            # Loads on the two HWDGE rings only. SWDGE costs ~2.5us of Q7
            # descriptor-gen per dma_start; HWDGE costs ~0.7us of sequencer
            # issue per dma_start, so batch into few, ordered transfers.
            # SP ring is free at ~0.1us -> e/d-path there, dc-paired so the
            # first encoder matmul fires at ~4us and warms the HAM gate.
            # ACT ring is blocked until ~8us by its activation-table load
            # -> wout/bout there (not needed until main MMs at ~13us).
            henc_sb = cpool.tile([128, DC, T], fp32, tag="henc")
            wenc_sb = cpool.tile([128, DC, JC, 128], fp32, tag="wenc")
            for h in range(2):
                nc.sync.dma_start(
                    wenc_sb[:, 2 * h : 2 * h + 2, :, :], wenc[:, 2 * h : 2 * h + 2, :, :])
                nc.sync.dma_start(
                    henc_sb[:, 2 * h : 2 * h + 2, :], henc[:, 2 * h : 2 * h + 2, :])
            benc_sb = cpool.tile([128, JC], fp32, tag="benc")
            nc.sync.dma_start(benc_sb[:], benc[:])
            wdec_sb = cpool.tile([128, DC, JC, 128], fp32, tag="wdec")
            nc.sync.dma_start(wdec_sb[:], wdec[:])
            hdec_sb = cpool.tile([128, DC, U], fp32, tag="hdec")
            nc.sync.dma_start(hdec_sb[:], hdec[:])
            wout_sb = cpool.tile([128, JC, V], bf16, tag="wout")
            nc.scalar.dma_start(wout_sb[:], wout[:])
            bout_sb = cpool.tile([128, V], fp32, tag="bout")
            nc.scalar.dma_start(bout_sb[:], bout[:])

            # --- prologue: e_T (j, t) and d_T (j, u) ---
            e_sb = wpool.tile([128, JC, T], fp32, tag="e")
            d_sb = wpool.tile([128, JC, U], fp32, tag="d")
            for jc in range(JC):
                ps_e = psepool.tile([128, T], fp32, tag="pse")
                for dc in range(DC):
                    nc.tensor.matmul(
                        ps_e[:],
                        wenc_sb[:, dc, jc, :],
                        henc_sb[:, dc, :],
                        start=(dc == 0),
                        stop=(dc == DC - 1),
                    )
                # evacuate with the encoder bias folded in (per-partition bias)
                nc.scalar.activation(
                    e_sb[:, jc, :], ps_e[:],
                    mybir.ActivationFunctionType.Identity,
                    bias=benc_sb[:, jc : jc + 1],
                )
            for jc in range(JC):
                ps_d = psepool.tile([128, U], fp32, tag="pse")
                for dc in range(DC):
                    nc.tensor.matmul(
                        ps_d[:],
                        wdec_sb[:, dc, jc, :],
                        hdec_sb[:, dc, :],
                        start=(dc == 0),
                        stop=(dc == DC - 1),
                    )
                nc.vector.tensor_copy(d_sb[:, jc, :], ps_d[:])

            # --- main loop: one u at a time ---
            for u in range(U):
                z_u = zpool.tile([128, JC, T], bf16, tag="z")
                for jc in range(JC):
                    # z = tanh(e + d_u); d_u enters as the per-partition bias
                    nc.scalar.activation(
                        z_u[:, jc, :], e_sb[:, jc, :],
                        mybir.ActivationFunctionType.Tanh,
                        bias=d_sb[:, jc, u : u + 1],
                    )
                for th in range(TH):
                    ps = pspool.tile([128, V], fp32, tag="ps")
                    for jc in range(JC):
                        lhsT = z_u[:, jc, th * 128 : (th + 1) * 128]
                        nc.tensor.matmul(
                            ps[:, 0:512], lhsT, wout_sb[:, jc, 0:512],
                            start=(jc == 0), stop=(jc == JC - 1),
                        )
                        nc.tensor.matmul(
                            ps[:, 512:1024], lhsT, wout_sb[:, jc, 512:1024],
                            start=(jc == 0), stop=(jc == JC - 1),
                        )
                    ot = opool.tile([128, V], fp32, tag="ot")
                    nc.vector.tensor_add(ot[:], ps[:], bout_sb[:])
                    nc.sync.dma_start(out[th * 128 : (th + 1) * 128, u, :], ot[:])

    _split_multiwait(nc)
    return nc


def _split_multiwait(nc, max_waits=1):
    """The pinned walrus rejects instructions carrying >~2 sem waits
    ("Too many sync wait commands"). Split extras into single-wait NoOps
    executed immediately before the offending instruction on the same engine."""
    import concourse.mybir as mybir

    for fn in nc.m.functions:
        for blk in fn.blocks:
            newlist = []
            for inst in blk.instructions:
                si = getattr(inst, "sync_info", None)
                if si is not None and si.on_wait and len(si.on_wait) > max_waits:
                    waits = list(si.on_wait)
                    for j, w in enumerate(waits[max_waits:]):
                        nop = mybir.InstNoOp(
                            name=f"{inst.name}-ws{j}", ins=[], outs=[],
                            sync_info=mybir.SyncInfo(on_wait=[w], on_update=[]),
                        )
                        nop.engine = inst.engine
                        newlist.append(nop)
                    si.on_wait = waits[:max_waits]
                newlist.append(inst)
            blk.instructions = newlist


def _prep_inputs(h_enc, h_dec, W_enc, b_enc, W_dec, W_out, b_out):
    """Host-side relayout into the SBUF layouts the kernel expects."""
    f32 = np.float32
    h_enc = np.asarray(h_enc, f32)
    h_dec = np.asarray(h_dec, f32)
    W_enc = np.asarray(W_enc, f32)
    W_dec = np.asarray(W_dec, f32)
    W_out = np.asarray(W_out, f32)
    b_enc = np.asarray(b_enc, f32)
    b_out = np.asarray(b_out, f32)

    wenc = np.ascontiguousarray(
        W_enc.reshape(JC, 128, DC, 128).transpose(3, 2, 0, 1))  # [p, dc, jc, m]
    wdec = np.ascontiguousarray(
        W_dec.reshape(JC, 128, DC, 128).transpose(3, 2, 0, 1))
    wout = np.ascontiguousarray(
        W_out.reshape(V, JC, 128).transpose(2, 1, 0)).astype(ml_dtypes.bfloat16)
    benc = np.ascontiguousarray(b_enc.reshape(JC, 128).T)
    bout = np.ascontiguousarray(np.broadcast_to(b_out[None, :], (128, V)))

    in_maps = []
    for b in range(B):
        A = h_enc[b, :, 0, :]  # (T, D)
        henc = np.ascontiguousarray(A.reshape(T, DC, 128).transpose(2, 1, 0))
        Bm = h_dec[b, 0, :, :]  # (U, D)
        hdec = np.ascontiguousarray(Bm.reshape(U, DC, 128).transpose(2, 1, 0))
        in_maps.append({
            "henc": henc, "hdec": hdec,
            "wenc": wenc, "wdec": wdec, "wout": wout,
            "benc": benc, "bout": bout,
        })
    return in_maps


def run(inputs, trace=False, trace_cores=None):
    """Build+compile (cached), run on 8 cores, return (output, BassKernelResults)."""
    from concourse.bass_utils import run_bass_kernel_spmd

    global _compiled
    if _compiled is None:
        _compiled = _build_kernel()
    nc = _compiled

    in_maps = _prep_inputs(**inputs)
    res = run_bass_kernel_spmd(
        nc, in_maps, core_ids=list(range(B)), trace=trace,
        trace_cores=trace_cores,
    )
    outp = np.stack([np.asarray(res.results[b]["out"]) for b in range(B)], axis=0)
    return outp.astype(np.float32, copy=False), res


def kernel(**inputs):
    outp, _ = run(inputs, trace=False)
    return outp
